# revision 7
# baseline (speedup 1.0000x reference)
"""Trainium2 Bass kernel for nn_Blur: per-sample 3D PSF blur (grouped conv3d).

Strategy
--------
The PSF  K[z,i,j] = (1 - exp(-alpha * ax[z] * lat[i,j])) / S  with
lat[i,j] = g[i]*g[j]/(2*pi*bxy^2) is, by Taylor expansion of 1-exp(-t),
an *exact* (to fp32) rank-4 CP tensor:

    K[z,i,j] = sum_m A[z,m] * U[i,m] * U[j,m],   m = 1..4
    A[z,m] = (-1)^(m+1) c[z]^m / m! / S,  U[i,m] = g[i]^m,
    c[z] = alpha*ax[z]/(2*pi*bxy^2)

so the 3D conv factorizes into 1D convs: y-conv, x-conv, then a z-conv
fused with the rank summation.  All three stages are PE matmuls (bf16
operands, fp32 PSUM):

  SA  per z-plane: PE-transpose input [x,y] -> [y,x] tiles
  SB  y-conv: data-stationary matmul, moving = Toeplitz(U_m on y)
  SC  x-conv: Toeplitz(U_m on x)-stationary matmul
  SD  PE-transpose to put (m,z) on partitions (4 ranks x 32 z = 128)
  SE  z-conv + rank sum: one [128 -> 32] dense banded stationary

Sharding: 8 cores = 4 samples x 2 x-halves (halo 7 in x, handled by
host-side padding).  No cross-core communication.
"""

import math
import sys

import numpy as np

for p in ("/opt/trn_rl_repo", "/root/.axon_site/_ro/trn_rl_repo"):
    if p not in sys.path:
        sys.path.append(p)

# geometry (hardcoded for this problem)
B = 4
Z, X, Y = 32, 192, 192
KZ, KT = 9, 15          # z taps; x/y taps
XH = X // 2             # 96 output x per core
XIN = XH + KT - 1       # 110 input x rows per core
YIN = Y + KT - 1        # 206 padded y
R = 4                   # CP rank (Taylor order)
NCORES = 8

_CACHE = {}


def _taylor_factors(bet_xy, bet_z, alpha):
    """Per-sample CP factors (A[9,R], U[15,R]) of the globally-normalized PSF."""
    zd = np.abs(np.arange(KZ) - KZ // 2).astype(np.float64)
    xd = np.abs(np.arange(KT) - KT // 2).astype(np.float64)
    dp = xd[:, None] ** 2 + xd[None, :] ** 2
    S = 0.0
    for b in range(B):
        bxy, bz, al = float(bet_xy[b]), float(bet_z[b]), float(alpha[b])
        lat = np.exp(-dp / (2 * bxy**2)) / (2 * np.pi * bxy**2)
        ax = np.exp(-zd**2 / (2 * bz**2)) / (np.sqrt(2 * np.pi) * bz)
        S += (1.0 - np.exp(-al * lat[None] * ax[:, None, None])).sum()
    facs = []
    for b in range(B):
        bxy, bz, al = float(bet_xy[b]), float(bet_z[b]), float(alpha[b])
        g = np.exp(-xd**2 / (2 * bxy**2))
        ax = np.exp(-zd**2 / (2 * bz**2)) / (np.sqrt(2 * np.pi) * bz)
        c = al * ax / (2 * np.pi * bxy**2)
        A = np.stack(
            [(-1) ** (m + 1) * c**m / math.factorial(m) / S for m in range(1, R + 1)], 1
        )
        U = np.stack([g**m for m in range(1, R + 1)], 1)
        facs.append((A.astype(np.float32), U.astype(np.float32)))
    return facs


def _build_mats(A, U):
    """Device weight matrices for one sample (fp32; cast to bf16 on load)."""
    ty0 = np.zeros((R, 128, 114), np.float32)
    ty1 = np.zeros((R, 128, 78), np.float32)
    tx = np.zeros((R, XIN, XH), np.float32)
    zm = np.zeros((128, Z), np.float32)
    for m in range(R):
        for yp in range(128):
            for yo in range(114):
                j = yp - yo
                if 0 <= j < KT:
                    ty0[m, yp, yo] = U[j, m]
            for yo in range(78):
                j = yp - yo - 36  # global y_in = 78+yp, y_out = 114+yo
                if 0 <= j < KT:
                    ty1[m, yp, yo] = U[j, m]
        for i in range(XIN):
            for o in range(max(0, i - KT + 1), min(XH, i + 1)):
                tx[m, i, o] = U[i - o, m]
        for zi in range(Z):
            for zo in range(max(0, zi - 4), min(Z, zi + 5)):
                zm[m * Z + zi, zo] = A[zi - zo + 4, m]
    return ty0, ty1, tx, zm


def _build_program(reps=1):
    import concourse.mybir as mybir
    import concourse.tile as tile
    from concourse import bacc
    from concourse.masks import make_identity

    F32, BF16 = mybir.dt.float32, mybir.dt.bfloat16

    nc = bacc.Bacc("TRN2", target_bir_lowering=False, debug=False, num_devices=NCORES)

    xin_d = nc.dram_tensor("xin", [Z, XIN, YIN], F32, kind="ExternalInput")
    ty0_d = nc.dram_tensor("ty0", [R, 128, 114], F32, kind="ExternalInput")
    ty1_d = nc.dram_tensor("ty1", [R, 128, 78], F32, kind="ExternalInput")
    tx_d = nc.dram_tensor("tx", [R, XIN, XH], F32, kind="ExternalInput")
    zm_d = nc.dram_tensor("zm", [128, Z], F32, kind="ExternalInput")
    out_d = nc.dram_tensor("out", [Z, XH, Y], F32, kind="ExternalOutput")

    with tile.TileContext(nc) as tc:
        with (
            tc.tile_pool(name="consts", bufs=1) as consts,
            tc.tile_pool(name="persist", bufs=1) as persist,
            tc.tile_pool(name="planes", bufs=4) as planes,
            tc.tile_pool(name="tpose", bufs=8) as tpose,
            tc.tile_pool(name="psum", bufs=4, space="PSUM") as psp,
        ):
            ident = consts.tile([128, 128], BF16)
            make_identity(nc, ident[:])
            ty0 = [consts.tile([128, 114], BF16, name=f"ty0_{m}") for m in range(R)]
            ty1 = [consts.tile([128, 78], BF16, name=f"ty1_{m}") for m in range(R)]
            tx = [consts.tile([XIN, XH], BF16, name=f"tx_{m}") for m in range(R)]
            for m in range(R):
                nc.gpsimd.dma_start(out=ty0[m][:], in_=ty0_d[m])
                nc.gpsimd.dma_start(out=ty1[m][:], in_=ty1_d[m])
                nc.gpsimd.dma_start(out=tx[m][:], in_=tx_d[m])
            zmt = consts.tile([128, Z], BF16)
            nc.gpsimd.dma_start(out=zmt[:], in_=zm_d[:])

            # reps>1 (timing only): repeat the whole pipeline sequentially in
            # one NEFF; same-tag persistent tiles serialize the reps.
            for _rep in range(reps):
                W = [
                    persist.tile([XIN, Z * Y], BF16, name=f"w_{m}", tag=f"w_{m}")
                    for m in range(R)
                ]
                Xt = persist.tile([XH, R * Z * Y], BF16, tag="xt")
                Wz = persist.tile([128, Y * XH], BF16, tag="wz")
                Out = persist.tile([128, 24 * Y], F32, tag="outt")

                # SA + SB per input z-plane
                for zp in range(Z):
                    pl = planes.tile([XIN, YIN], BF16, tag="pl")
                    nc.gpsimd.dma_start(out=pl[:], in_=xin_d[zp])
                    psa = psp.tile([128, 512], BF16, tag="pst")
                    nc.tensor.transpose(
                        psa[:128, :XIN], pl[:, 0:128], ident[:XIN, :XIN]
                    )
                    ta = tpose.tile([128, XIN], BF16, tag="ta")
                    nc.any.tensor_copy(out=ta[:], in_=psa[:128, :XIN])
                    psb = psp.tile([128, 512], BF16, tag="pst")
                    nc.tensor.transpose(
                        psb[:128, :XIN], pl[:, 78:206], ident[:XIN, :XIN]
                    )
                    tb = tpose.tile([128, XIN], BF16, tag="tb")
                    nc.any.tensor_copy(out=tb[:], in_=psb[:128, :XIN])
                    for m in range(R):
                        ps = psp.tile([128, 512], F32, tag="ps")
                        nc.tensor.matmul(ps[:XIN, 0:114], ta[:], ty0[m][:])
                        nc.tensor.matmul(ps[:XIN, 114:192], tb[:], ty1[m][:])
                        nc.any.tensor_copy(
                            out=W[m][:, zp * Y : (zp + 1) * Y], in_=ps[:XIN, 0:Y]
                        )

                # SC: x-conv, 12 chunks of 512 per rank
                for m in range(R):
                    for c in range(Z * Y // 512):
                        ps = psp.tile([128, 512], F32, tag="ps")
                        nc.tensor.matmul(
                            ps[:XH, :], tx[m][:], W[m][:, c * 512 : (c + 1) * 512]
                        )
                        nc.any.tensor_copy(
                            out=Xt[:, m * Z * Y + c * 512 : m * Z * Y + (c + 1) * 512],
                            in_=ps[:XH, :],
                        )

                # SD: permute to (m,z) partitions
                Xr = Xt[:].rearrange("p (m z y) -> p m z y", m=R, z=Z, y=Y)
                for yv in range(Y):
                    pst = psp.tile([128, 512], BF16, tag="pst")
                    nc.tensor.transpose(
                        pst[:128, :XH], Xr[:, :, :, yv : yv + 1], ident[:XH, :XH]
                    )
                    nc.any.tensor_copy(
                        out=Wz[:, yv * XH : (yv + 1) * XH], in_=pst[:128, :XH]
                    )

                # SE: z-conv + rank sum; output chunks of 2 x-rows
                Wr = Wz[:].rearrange("p (y x) -> p x y", y=Y, x=XH)
                for xp in range(XH // 2):
                    x0 = 2 * xp
                    xg, xl = x0 // 24, x0 % 24
                    ps = psp.tile([128, 512], F32, tag="ps")
                    nc.tensor.matmul(ps[:Z, : 2 * Y], zmt[:], Wr[:, x0 : x0 + 2, :])
                    nc.any.tensor_copy(
                        out=Out[xg * Z : (xg + 1) * Z, xl * Y : (xl + 2) * Y],
                        in_=ps[:Z, : 2 * Y],
                    )

                # out DRAM [z, x, y] <- Out [(xg z), (xl y)]; one DMA per xg
                for xg in range(4):
                    nc.sync.dma_start(
                        out=out_d[:, xg * 24 : (xg + 1) * 24, :],
                        in_=Out[xg * Z : (xg + 1) * Z, :],
                    )

    nc.compile()
    return nc


def _make_in_maps(x, bet_xy, bet_z, alpha):
    facs = _taylor_factors(np.asarray(bet_xy), np.asarray(bet_z), np.asarray(alpha))
    in_maps = []
    for c in range(NCORES):
        b, xh = c // 2, c % 2
        A, U = facs[b]
        ty0, ty1, tx, zm = _build_mats(A, U)
        xpad = np.zeros((Z, XIN, YIN), np.float32)
        x0 = XH * xh - 7
        lo, hi = max(0, x0), min(X, x0 + XIN)
        xpad[:, lo - x0 : hi - x0, 7 : 7 + Y] = x[b, 0, :, lo:hi, :]
        in_maps.append({"xin": xpad, "ty0": ty0, "ty1": ty1, "tx": tx, "zm": zm})
    return in_maps


def kernel(x, bet_xy, bet_z, alpha):
    from concourse.bass_utils import run_bass_kernel_spmd

    x = np.asarray(x, dtype=np.float32)

    if "nc" not in _CACHE:
        _CACHE["nc"] = _build_program()
    nc = _CACHE["nc"]

    in_maps = _make_in_maps(x, bet_xy, bet_z, alpha)
    res = run_bass_kernel_spmd(nc, in_maps, list(range(NCORES))).results

    out = np.empty((B, 1, Z, X, Y), np.float32)
    for c in range(NCORES):
        b, xh = c // 2, c % 2
        out[b, 0, :, XH * xh : XH * (xh + 1), :] = res[c]["out"]
    return out
